# revision 31
# baseline (speedup 1.0000x reference)
"""Segment-mean (scatter-add + divide) of face features onto vertices, on 8
Trainium2 NeuronCores.

Problem: out[v] = mean over corners c with faces[c]==v of
face_features.reshape(3F, 192)[c], with F=500k faces, V=250k vertices, D=192.

Strategy (window-sharded, no collectives, DMA-roofline oriented):
  - The vertex space is cut into 128-vertex aligned windows. Host sorts the
    1.5M corner indices by vertex id (index-space metadata only). Windows are
    snake-paired (largest with smallest) into SLOTS of two windows whose
    corner streams are packed contiguously — a chunk at the pair boundary
    carries corners of both windows. Corners' in-slot relative ids are
    0..127 for the first window, 128..255 for the second, so each window's
    one-hot (built against its own iota slice) automatically zeroes the
    other window's corners. This cuts the per-window ceil(128) padding from
    ~8.4% to ~0.8% of the value stream.
  - Slot pairs are dealt to (core, slot) so the SPMD program (chunk count
    per slot, per-window chunk ranges) is the max/min envelope over the 8
    cores; envelope-only chunks contribute zero rows, so correctness is
    unconditional.
  - Corner VALUES are laid out per core in sorted, 128-partition-transposed,
    DMA-contiguous order as plain bf16 (~0.2% rounding, 10x inside the 2e-2
    gate) — half the HBM traffic of an fp32-accurate encoding, and the
    matmuls run at full bf16 PE rate.
  - Per (slot, window), the one-hot [corner, vertex-in-window] is built
    either on the Vector engine (per-chunk tensor_scalar(is_equal): the
    window's iota slice vs the chunk's relative ids as a per-partition fp32
    scalar) or on the GpSimd engine (one local_scatter of a ones-row into
    the window's [128, range*128] one-hot using pre-offset int16 indices,
    negatives ignored). Windows are dealt greedily to the two engines so
    neither becomes the bottleneck. The TensorEngine accumulates
    onehot.T @ vals[128, 192] into PSUM.
  - Per-vertex reciprocal hit counts are computed on the host (a byproduct
    of planning) and shipped as a tiny [P, nwin_core] fp32 tensor; the
    Scalar engine applies them while copying PSUM->SBUF, casting to bf16.
  - Results are batched per slab in window-contiguous [P, windows*D] layout
    and streamed to DRAM as large per-partition-contiguous stores; the host
    transposes/casts back to the full fp32 output.

Dummy (padding) corner slots carry relative id -1 so no window's one-hot
selects them.
"""

import numpy as np

P = 128          # partitions / window size
G = 2            # windows per slot
D = 192          # feature dim
NCORES = 8
SLAB_CHUNK_BUDGET = 96   # chunks per steady-state DMA slab (~4.7 MB loads)
SLAB_RAMP = (16, 32, 64)  # smaller first slabs so compute starts early

_prog_cache = {}


def _even(k):
    """local_scatter needs an even index count per window."""
    return k + (k & 1)


def _deal_engines(rngs):
    """Greedy slot-level deal of one-hot builds between DVE (tensor_scalar
    is_equal, ~195 ns/chunk) and Pool (local_scatter, ~177 ns/chunk + launch).
    Returns a bool list: True = slot's one-hots build on DVE.
    Host prep and program builder must agree on this deal, so it lives here.
    """
    on_dve = []
    acc_v = acc_g = 0.0
    for rt in rngs:
        cost_v = sum(195.0 * r + 45.0 for r in rt)
        cost_g = sum(177.0 * _even(r) + 250.0 for r in rt)
        if acc_v + cost_v <= acc_g + cost_g:
            acc_v += cost_v
            on_dve.append(True)
        else:
            acc_g += cost_g
            on_dve.append(False)
    return on_dve


def _plan_slabs(ks):
    """Group consecutive slots into slabs; first slabs are smaller so the
    pipeline fills quickly, and the last ones shrink so the drain is short."""
    total = int(sum(ks))
    slabs = []  # (slot_start, n_slots, n_chunks)
    s = 0
    done = 0
    while s < len(ks):
        budget = SLAB_RAMP[len(slabs)] if len(slabs) < len(SLAB_RAMP) else SLAB_CHUNK_BUDGET
        remaining = total - done
        if remaining <= SLAB_CHUNK_BUDGET + 64:
            budget = min(budget, max(8, remaining // 3))
        n_slots = 0
        n_chunks = 0
        while s + n_slots < len(ks) and n_chunks + ks[s + n_slots] <= budget:
            n_chunks += ks[s + n_slots]
            n_slots += 1
        if n_slots == 0:  # oversized slot gets its own slab
            n_slots, n_chunks = 1, int(ks[s])
        slabs.append((s, n_slots, n_chunks))
        s += n_slots
        done += n_chunks
    return slabs


def _build_program(ks, r0s, r1s):
    """ks[t]: chunks of slot t; r0s/r1s[t][i]: window i's chunk range."""
    import concourse.bacc as bacc
    import concourse.tile as tile
    from concourse import mybir

    nt = len(ks)
    c = int(sum(ks))
    cs = np.concatenate([[0], np.cumsum(ks)]).astype(int)
    rngs = [[int(r1s[t][i] - r0s[t][i]) for i in range(G)] for t in range(nt)]
    e_rngs = [[_even(r) for r in rt] for rt in rngs]
    on_dve = _deal_engines(rngs)
    # idxr covers only DVE slots' chunks; idxi only Pool slots' index blocks
    cr = np.concatenate(
        [[0], np.cumsum([ks[t] if on_dve[t] else 0 for t in range(nt)])]
    ).astype(int)
    c1 = int(cr[-1])
    ci2 = np.concatenate(
        [[0], np.cumsum([0 if on_dve[t] else sum(e_rngs[t]) for t in range(nt)])]
    ).astype(int)
    c2 = int(ci2[-1])
    slabs = _plan_slabs(ks)
    max_slab_chunks = max(sl[2] for sl in slabs)
    max_slab_slots = max(sl[1] for sl in slabs)
    max_rng2 = max(max(e) for e in e_rngs)
    f32 = mybir.dt.float32
    bf16 = mybir.dt.bfloat16
    i16 = mybir.dt.int16

    nc = bacc.Bacc(None, target_bir_lowering=False)
    vals_in = nc.declare_dram_parameter("vals", [P, c * D], bf16, isOutput=False)
    idxr_in = nc.declare_dram_parameter("idxr", [P, c1], f32, isOutput=False)
    idxi_in = nc.declare_dram_parameter("idxi", [P, c2], i16, isOutput=False)
    iota_in = nc.declare_dram_parameter("iota", [P, G * P], bf16, isOutput=False)
    rec_in = nc.declare_dram_parameter("rec", [P, nt * G], f32, isOutput=False)
    out_dram = nc.declare_dram_parameter("out", [P, nt * G * D], bf16, isOutput=True)

    with tile.TileContext(nc) as tc:
        with (
            tc.tile_pool(name="const", bufs=1) as constp,
            tc.tile_pool(name="slab", bufs=4) as slabp,
            tc.tile_pool(name="oh", bufs=4) as ohp,
            tc.tile_pool(name="ot", bufs=3) as otp,
            tc.tile_pool(name="ps", bufs=8, space="PSUM") as psump,
        ):
            iota_t = constp.tile([P, G * P], bf16)
            nc.sync.dma_start(out=iota_t[:], in_=iota_in[:])
            rec_t = constp.tile([P, nt * G], f32)
            nc.sync.dma_start(out=rec_t[:], in_=rec_in[:])
            idxr_t = constp.tile([P, c1], f32)
            nc.scalar.dma_start(out=idxr_t[:], in_=idxr_in[:])
            idxi_t = constp.tile([P, c2], i16)
            nc.sync.dma_start(out=idxi_t[:], in_=idxi_in[:])
            ones_t = constp.tile([P, max_rng2], bf16)
            nc.gpsimd.memset(ones_t[:], 1.0)

            for si, (s0, n_slots, n_chunks) in enumerate(slabs):
                base_chunk = int(cs[s0])
                slab = slabp.tile([P, max_slab_chunks * D], bf16, tag="slab")
                ldeng = nc.sync if si % 2 == 0 else nc.scalar
                ldeng.dma_start(
                    out=slab[:, : n_chunks * D],
                    in_=vals_in[:, base_chunk * D : (base_chunk + n_chunks) * D],
                )
                oslab = otp.tile([P, max_slab_slots * G * D], bf16, tag="ot")
                for tt in range(n_slots):
                    t = s0 + tt
                    c0 = int(cs[t])       # global chunk index of slot start
                    l0 = c0 - base_chunk  # chunk offset within slab
                    for i in range(G):
                        r0, r1 = int(r0s[t][i]), int(r1s[t][i])
                        rng = r1 - r0
                        rng2 = _even(rng)
                        oh = ohp.tile([P, max_rng2, P], bf16, tag="oh")
                        if on_dve[t]:
                            cb = int(cr[t])
                            for j in range(r0, r1):
                                nc.vector.tensor_scalar(
                                    out=oh[:, j - r0, :],
                                    in0=iota_t[:, i * P : (i + 1) * P],
                                    scalar1=idxr_t[:, cb + j : cb + j + 1],
                                    scalar2=None,
                                    op0=mybir.AluOpType.is_equal,
                                )
                        else:
                            i0 = int(ci2[t]) + (e_rngs[t][0] if i == 1 else 0)
                            nc.gpsimd.local_scatter(
                                out_ap=oh[:, :rng2, :],
                                data_ap=ones_t[:, :rng2],
                                idxs_ap=idxi_t[:, i0 : i0 + rng2],
                                channels=P,
                                num_elems=rng2 * P,
                                num_idxs=rng2,
                            )
                        ps = psump.tile([P, D], f32)
                        for j in range(r0, r1):
                            off = (l0 + j) * D
                            nc.tensor.matmul(
                                out=ps[:],
                                lhsT=oh[:, j - r0, :],
                                rhs=slab[:, off : off + D],
                                start=(j == r0),
                                stop=(j == r1 - 1),
                            )
                        w = t * G + i
                        nc.scalar.activation(
                            out=oslab[:, (tt * G + i) * D : (tt * G + i + 1) * D],
                            in_=ps[:],
                            func=mybir.ActivationFunctionType.Copy,
                            scale=rec_t[:, w : w + 1],
                        )
                # store on the HWDGE queue opposite this slab's load so the
                # two rings stay co-busy
                steng = nc.scalar if si % 2 == 0 else nc.sync
                steng.dma_start(
                    out=out_dram[:, s0 * G * D : (s0 + n_slots) * G * D],
                    in_=oslab[:, : n_slots * G * D],
                )
    nc.compile()
    return nc


def get_program(ks, r0s, r1s):
    key = (
        tuple(int(k) for k in ks),
        tuple(tuple(int(x) for x in r) for r in r0s),
        tuple(tuple(int(x) for x in r) for r in r1s),
    )
    if key not in _prog_cache:
        _prog_cache[key] = _build_program(*key)
    return _prog_cache[key]


def _plan(idx, vcount):
    """Pair windows into slots, deal slots to cores, derive the SPMD program
    envelope (chunks per slot, per-window chunk ranges)."""
    nwin_real = (vcount + P - 1) // P
    nwin = -(-nwin_real // (NCORES * G)) * (NCORES * G)
    counts = np.bincount(idx, minlength=nwin * P)
    w = counts.reshape(nwin, P).sum(1)

    # snake-pair sorted windows into slots of G to equalize slot totals
    o = np.argsort(-w, kind="stable")
    nslot = nwin // G
    sl = np.empty((G, nslot), dtype=np.int64)
    for i in range(G):
        seg = o[i * nslot : (i + 1) * nslot]
        sl[i] = seg if i % 2 == 0 else seg[::-1]
    slots = sl.T                       # [nslot, G] window ids
    tot = w[slots].sum(1)

    # deal slots to cores by sorted totals
    o2 = np.argsort(-tot, kind="stable")
    nt = nslot // NCORES
    sgroups = o2.reshape(nt, NCORES)   # [nt, j] -> slot id

    kcore = np.maximum(-(-tot // P), 1).astype(np.int64)       # per-slot chunks
    ks = kcore[sgroups].max(1)                                  # program chunks

    # per-window chunk ranges within the slot stream (per slot, then envelope)
    pre = np.concatenate(
        [np.zeros((nslot, 1), dtype=np.int64), np.cumsum(w[slots], 1)], axis=1
    )
    kc = kcore[:, None]
    r0 = np.minimum(pre[:, :-1] // P, kc - 1)                   # [nslot, G]
    r1 = np.minimum(np.maximum(-(-pre[:, 1:] // P), r0 + 1), kc)
    r0p = r0[sgroups].min(1)                                    # [nt, G]
    r1p = r1[sgroups].max(1)
    return slots, sgroups, ks, r0p, r1p, counts, w


def _host_prep(vals_flat, idx, slots, sgroups, ks, r0p, r1p, counts, w):
    import ml_dtypes

    bf16 = ml_dtypes.bfloat16
    nt = sgroups.shape[0]
    nslot, _ = slots.shape
    nwin = nslot * G
    c = int(ks.sum())
    cs = np.concatenate([[0], np.cumsum(ks)]).astype(np.int64)
    rngs = r1p - r0p                                            # [nt, G]
    e_rngs = rngs + (rngs & 1)
    on_dve = np.array(_deal_engines(rngs.tolist()))
    cr = np.concatenate([[0], np.cumsum(np.where(on_dve, ks, 0))]).astype(
        np.int64
    )
    c1 = int(cr[-1])
    ci2 = np.concatenate(
        [[0], np.cumsum(np.where(on_dve, 0, e_rngs.sum(1)))]
    ).astype(np.int64)
    c2 = int(ci2[-1])

    # per-window location: (core, slot-row, window-pos)
    slot_core = np.empty(nslot, dtype=np.int64)
    slot_row = np.empty(nslot, dtype=np.int64)
    for j in range(NCORES):
        slot_core[sgroups[:, j]] = j
        slot_row[sgroups[:, j]] = np.arange(nt)
    win_slot = np.empty(nwin, dtype=np.int64)   # window -> slot id
    win_pos = np.empty(nwin, dtype=np.int64)    # window -> position in slot
    for i in range(G):
        win_slot[slots[:, i]] = np.arange(nslot)
        win_pos[slots[:, i]] = i
    # corner offset of each window within its slot's packed stream
    pre = np.concatenate(
        [np.zeros((nslot, 1), dtype=np.int64), np.cumsum(w[slots], 1)], axis=1
    )
    win_off = np.empty(nwin, dtype=np.int64)
    for i in range(G):
        win_off[slots[:, i]] = pre[:, i]

    # sorted corner stream
    order = np.argsort(idx, kind="stable")
    idx_s = idx[order]
    wod = idx_s >> 7                                  # window of each corner
    win_start = np.searchsorted(idx_s, np.arange(nwin, dtype=np.int64) * P)
    pos_in_win = np.arange(len(idx_s), dtype=np.int64) - win_start[wod]

    csl = win_slot[wod]                               # corner's slot id
    crow = slot_row[csl]                              # slot row on its core
    ccore = slot_core[csl]
    cpos = win_pos[wod]                               # window pos in slot
    pos_slot = win_off[wod] + pos_in_win              # corner pos in slot stream
    chunk_in_slot = pos_slot >> 7
    corner_part = pos_slot & (P - 1)
    corner_chunk = cs[crow] + chunk_in_slot
    corner_dve = on_dve[crow]
    corner_rel = (cpos * P + (idx_s & (P - 1))).astype(np.float32)
    corner_cr = cr[crow] + chunk_in_slot              # idxr col (DVE slots)
    # int16 scatter index into the window's one-hot range (Pool slots)
    corner_r0 = r0p[crow, cpos]
    corner_sidx = ((chunk_in_slot - corner_r0) * P + (idx_s & (P - 1))).astype(
        np.int16
    )
    corner_c2 = (
        ci2[crow] + np.where(cpos == 1, e_rngs[crow, 0], 0)
        + (chunk_in_slot - corner_r0)
    )

    recip = (1.0 / np.maximum(counts, 1)).astype(np.float32).reshape(nwin, P)
    # rec laid out [P, nt*G] in (slot-row, window-pos) order per core
    win_of = np.empty((nt, G, NCORES), dtype=np.int64)
    for j in range(NCORES):
        win_of[:, :, j] = slots[sgroups[:, j]]

    iota = np.tile(np.arange(G * P, dtype=bf16), (P, 1))
    in_maps = []
    for j in range(NCORES):
        m = ccore == j
        mv = m & corner_dve
        mg = m & ~corner_dve
        gmap = np.zeros((P, c), dtype=np.int64)
        idxr = np.full((P, c1), -1.0, dtype=np.float32)
        idxi = np.full((P, c2), -1, dtype=np.int16)
        gmap[corner_part[m], corner_chunk[m]] = order[m]
        idxr[corner_part[mv], corner_cr[mv]] = corner_rel[mv]
        idxi[corner_part[mg], corner_c2[mg]] = corner_sidx[mg]

        vals2 = vals_flat[gmap].astype(bf16).reshape(P, c * D)
        rec = np.ascontiguousarray(
            recip[win_of[:, :, j].reshape(-1)].T
        )  # [P, nt*G]
        in_maps.append(
            {"vals": vals2, "idxr": idxr, "idxi": idxi, "iota": iota, "rec": rec}
        )
    return in_maps, win_of


def run(face_features, faces, vertex_count, trace=False, tmpdir=None):
    from concourse.bass_utils import run_bass_kernel_spmd

    vcount = int(vertex_count)
    ff = np.ascontiguousarray(np.asarray(face_features, dtype=np.float32))
    nf = ff.shape[0]
    vals_flat = ff.reshape(nf * 3, D)
    idx = np.asarray(faces).reshape(-1).astype(np.int64)
    assert idx.min() >= 0 and idx.max() < vcount, "face indices out of range"

    slots, sgroups, ks, r0p, r1p, counts, w = _plan(idx, vcount)
    nc = get_program(ks, r0p, r1p)
    in_maps, win_of = _host_prep(
        vals_flat, idx, slots, sgroups, ks, r0p, r1p, counts, w
    )
    kw = {}
    if trace:
        kw = dict(trace=True, tmpdir=tmpdir)
    res = run_bass_kernel_spmd(nc, in_maps, list(range(NCORES)), **kw)

    nt = sgroups.shape[0]
    nwin = slots.shape[0] * G
    out = np.empty((nwin * P, D), dtype=np.float32)
    out_w = out.reshape(nwin, P, D)
    for j in range(NCORES):
        r = np.asarray(res.results[j]["out"]).reshape(P, nt * G, D)
        out_w[win_of[:, :, j].reshape(-1)] = r.transpose(1, 0, 2).astype(
            np.float32
        )
    return out[:vcount], res


def kernel(face_features, faces, vertex_count):
    out, _ = run(face_features, faces, vertex_count)
    return out
